# revision 1
# baseline (speedup 1.0000x reference)
"""Elman RNN on 8 Trainium2 NeuronCores.

Strategy: time-shard T=512 across the 8 cores (64 owned steps each) and
exploit the contractivity of the relu recurrence: each core re-runs a
48-step burn-in from h=0 before its owned window, which converges to the
true hidden state to ~5e-7 relative error (fp32 noise floor); the first
24 burn-in steps feed bf16 x (their rounding error also contracts away).
Core 0 has no real predecessor steps; its burn-in input is a forcing
vector x* with W_x @ x* = -1e4, so relu clamps h to exactly 0 until its
window starts.

On-chip layout is transposed: the hidden state g = h^T lives as
(D=128 partitions, N=256 free). Per step:
  PE:   psum[:, step] += W_h^T.T @ g_prev      (xproj pre-filled per pair)
  ACT:  gA = relu(psum[:, nA] + b_x)           (batch half A)
  DVE:  gB = relu(psum[:, nB] + b_x)           (batch half B)
Owned steps: y^T = W_y^T.T @ g into PSUM (evacuated per 4-step quad on
DVE with b_y added as a per-partition bias), h^T DMA'd straight from the
g tiles. Both outputs are written transposed — (K, OWN*N) / (D, OWN*N) —
and the host untransposes during reassembly. This keeps the PE free of
transpose and bias matmuls (fp32 matmul/LDWEIGHTS are 2-pass on trn2,
so every avoided PE op counts double).
"""

import sys

if "/opt/trn_rl_repo" not in sys.path:
    sys.path.insert(0, "/opt/trn_rl_repo")

import numpy as np

T, N, C, D, K = 512, 256, 128, 128, 128
NCORES = 8
OWN = T // NCORES          # 64 owned timesteps per core
BURN = 48                  # burn-in steps (contraction reaches fp32 floor)
NBF = 24                   # leading burn-in steps fed bf16 x (errors contract)
S = OWN + BURN             # 112 recurrence steps per core
FORCE = 1.0e4
HALF = N // 2              # 128: batch half per relu chain
PF = 2                     # xproj prefetch depth, in pairs
BF_PAIRS = NBF // 2        # pairs taking the bf16 xproj path
OQ = OWN // 4              # owned quads (4-step output groups)

_prog_cache = {}


def _build_program(repeats=1, bench_internal=False):
    """bench_internal: big I/O tensors become device-internal scratch so
    per-call host staging vanishes — used only for device-time measurement."""
    from contextlib import ExitStack

    import concourse.tile as tile
    from concourse import bacc, mybir

    f32 = mybir.dt.float32
    bf = mybir.dt.bfloat16
    AF = mybir.ActivationFunctionType
    ALU = mybir.AluOpType

    nc = bacc.Bacc(
        "TRN2", target_bir_lowering=False, debug=False, num_devices=NCORES
    )
    big = "Internal" if bench_internal else None
    xT = nc.dram_tensor(
        "xT", [C, (S - NBF) * N], f32, kind=big or "ExternalInput"
    ).ap()
    xTb = nc.dram_tensor("xTb", [C, NBF * N], bf, kind=big or "ExternalInput").ap()
    wxb = nc.dram_tensor("wxb", [C, D], bf, kind="ExternalInput").ap()
    wxt = nc.dram_tensor("wxt", [C, D], f32, kind="ExternalInput").ap()
    wht = nc.dram_tensor("wht", [D, D], f32, kind="ExternalInput").ap()
    wyt = nc.dram_tensor("wyt", [D, K], f32, kind="ExternalInput").ap()
    bx = nc.dram_tensor("bx", [D, 1], f32, kind="ExternalInput").ap()
    by = nc.dram_tensor("by", [K, 1], f32, kind="ExternalInput").ap()
    y_o = nc.dram_tensor("y", [K, OWN * N], f32, kind=big or "ExternalOutput").ap()
    h_o = nc.dram_tensor("h", [D, OWN * N], f32, kind=big or "ExternalOutput").ap()
    dummy = None
    if bench_internal:
        dummy = nc.dram_tensor(
            "bench_out", [1, 1], f32, kind="ExternalOutput"
        ).ap()

    PAIRS = S // 2

    with ExitStack() as ctx:
        tc = ctx.enter_context(tile.TileContext(nc))
        consts = ctx.enter_context(tc.tile_pool(name="consts", bufs=1))
        xtp = ctx.enter_context(tc.tile_pool(name="xt", bufs=12))
        gqp = ctx.enter_context(tc.tile_pool(name="gq", bufs=5))
        styp = ctx.enter_context(tc.tile_pool(name="sty", bufs=4))
        recp = ctx.enter_context(tc.tile_pool(name="rec", bufs=3, space="PSUM"))
        yqp = ctx.enter_context(tc.tile_pool(name="yq", bufs=2, space="PSUM"))
        filp = ctx.enter_context(tc.tile_pool(name="fil", bufs=1, space="PSUM"))

        wxt_sb = consts.tile([C, D], f32)
        nc.sync.dma_start(wxt_sb[:], wxt)
        wxb_sb = consts.tile([C, D], bf)
        nc.sync.dma_start(wxb_sb[:], wxb)
        wht_sb = consts.tile([D, D], f32)
        nc.sync.dma_start(wht_sb[:], wht)
        wyt_sb = consts.tile([D, K], f32)
        nc.sync.dma_start(wyt_sb[:], wyt)
        bx_sb = consts.tile([D, 1], f32)
        nc.sync.dma_start(bx_sb[:], bx)
        by_sb = consts.tile([K, 1], f32)
        nc.sync.dma_start(by_sb[:], by)

        # HAM keep-warm filler: a 1-output-row bf16 matmul streaming 256
        # columns keeps the PE array "busy" through the per-step relu
        # windows, so the clock gate stays at 2.4 GHz instead of
        # re-throttling to 1.2 GHz (which doubles every real matmul).
        fill_w = consts.tile([D, 1], bf)
        nc.vector.memset(fill_w[:], 0.0)
        fill_x = consts.tile([D, 2 * N], bf)
        nc.vector.memset(fill_x[:], 0.0)
        fil_ps = filp.tile([1, 2 * N], f32)

        def emit_filler(ncols):
            nc.tensor.matmul(
                fil_ps[0:1, 0:ncols],
                fill_w[:],
                fill_x[:, 0:ncols],
                start=True,
                stop=True,
            )

        def emit_rep():
            rec_tiles = {}
            gq_tiles = {}
            yq_tiles = {}

            def emit_xproj(p):
                if p >= PAIRS:
                    return
                if p < BF_PAIRS:
                    xt_t = xtp.tile([C, 2 * N], bf, name="xtb_t", tag="xtb_t")
                    nc.sync.dma_start(
                        xt_t[:], xTb[:, p * 2 * N : (p + 1) * 2 * N]
                    )
                    lhs = wxb_sb
                else:
                    xt_t = xtp.tile([C, 2 * N], f32, name="xt_t", tag="xt_t")
                    q = p - BF_PAIRS
                    nc.sync.dma_start(
                        xt_t[:], xT[:, q * 2 * N : (q + 1) * 2 * N]
                    )
                    lhs = wxt_sb
                r = recp.tile([D, 2 * N], f32, name="rec_t", tag="rec_t")
                nc.tensor.matmul(r[:], lhs[:], xt_t[:], start=True, stop=True)
                rec_tiles[p] = r

            def emit_y(s, g_sl):
                """Deferred y^T matmul for step s, plus per-quad evac+DMA."""
                if s < BURN:
                    return
                o = s - BURN
                q, e = divmod(o, 4)
                if e == 0:
                    yq_tiles[q] = yqp.tile(
                        [K, 4 * N], f32, name="yq_t", tag="yq_t"
                    )
                yq = yq_tiles[q]
                # has_written clearing is per PSUM bank; the quad tile spans
                # two banks (slices 0-1 and 2-3), so the first slice landing
                # in each bank opens/closes that bank's group and the second
                # overwrites via the cleared has_written bits.
                opener = e % 2 == 0
                nc.tensor.matmul(
                    yq[:, e * N : (e + 1) * N],
                    wyt_sb[:],
                    g_sl,
                    start=opener,
                    stop=opener,
                    skip_group_check=not opener,
                )
                if e == 3:
                    sty = styp.tile([K, 4 * N], f32, name="sty_t", tag="sty_t")
                    # copy + per-partition b_y bias in one ACT op (keeps the
                    # evacuation off the DVE, which carries the B-half relus)
                    nc.scalar.activation(
                        sty[:], yq[:], AF.Identity, bias=by_sb[:]
                    )
                    nc.gpsimd.dma_start(
                        y_o[:, q * 4 * N : (q + 1) * 4 * N], sty[:]
                    )
                    del yq_tiles[q]

            for p in range(PF):
                emit_xproj(p)

            g_prev = None  # (tile, col_base) of previous step's g
            pend = None
            for s in range(S):
                p, e2 = divmod(s, 2)
                quad, e4 = divmod(s, 4)
                rec = rec_tiles[p]
                base = e2 * N
                if s > 0:
                    pt, pb = g_prev
                    nc.tensor.matmul(
                        rec[:, base : base + HALF],
                        wht_sb[:],
                        pt[:, pb : pb + HALF],
                        start=False,
                        stop=False,
                        skip_group_check=True,
                    )
                    nc.tensor.matmul(
                        rec[:, base + HALF : base + N],
                        wht_sb[:],
                        pt[:, pb + HALF : pb + N],
                        start=False,
                        stop=False,
                        skip_group_check=True,
                    )
                if e2 == 0:
                    emit_xproj(p + PF)
                if pend is not None:
                    emit_y(*pend)
                for _f in range(3 if s < BURN else 2):
                    emit_filler(2 * N)
                if e4 == 0:
                    gq_tiles[quad] = gqp.tile(
                        [D, 4 * N], f32, name="gq_t", tag="gq_t"
                    )
                gq = gq_tiles[quad]
                gb = e4 * N
                nc.scalar.activation(
                    gq[:, gb : gb + HALF],
                    rec[:, base : base + HALF],
                    AF.Relu,
                    bias=bx_sb[:],
                )
                nc.vector.tensor_scalar(
                    gq[:, gb + HALF : gb + N],
                    rec[:, base + HALF : base + N],
                    bx_sb[:],
                    0.0,
                    ALU.add,
                    ALU.max,
                )
                pend = (s, gq[:, gb : gb + N])
                g_prev = (gq, gb)
                if e4 == 3 and s >= BURN:
                    oq = quad - BURN // 4
                    nc.gpsimd.dma_start(
                        h_o[:, oq * 4 * N : (oq + 1) * 4 * N], gq[:]
                    )
                if e4 == 3 and quad - 1 in gq_tiles:
                    del gq_tiles[quad - 1]
                if e2 == 1:
                    rec_tiles.pop(p, None)
            emit_y(*pend)

        for _rep in range(repeats):
            emit_rep()

        if dummy is not None:
            nc.sync.dma_start(dummy, bx_sb[0:1, 0:1])

    nc.compile()
    return nc


def _get_program(repeats=1, bench_internal=False):
    key = (repeats, bench_internal)
    if key not in _prog_cache:
        _prog_cache[key] = _build_program(repeats, bench_internal)
    return _prog_cache[key]


def _prep_inputs(x, W_x, b_x, W_h, W_y, b_y):
    x = np.ascontiguousarray(x, np.float32)
    W_x = np.asarray(W_x, np.float32)
    b_x = np.asarray(b_x, np.float32)
    W_h = np.asarray(W_h, np.float32)
    W_y = np.asarray(W_y, np.float32)
    b_y = np.asarray(b_y, np.float32)

    # core-0 burn-in forcing vector: W_x @ x_star = -FORCE (relu clamps to 0)
    lam = np.linalg.solve(
        W_x.astype(np.float64) @ W_x.astype(np.float64).T,
        -FORCE * np.ones(D, np.float64),
    )
    x_star = (W_x.astype(np.float64).T @ lam).astype(np.float32)

    wxt = np.ascontiguousarray(W_x.T)                  # (C, D)
    wht = np.ascontiguousarray(W_h.T)                  # (D, D)
    wyt = np.ascontiguousarray(W_y.T)                  # (D, K)
    bxc = np.ascontiguousarray(b_x[:, None])           # (D, 1)
    byc = np.ascontiguousarray(b_y[:, None])           # (K, 1)

    import ml_dtypes

    wxb = W_x.T.astype(ml_dtypes.bfloat16)

    in_maps = []
    for core in range(NCORES):
        t0 = core * OWN - BURN
        xw = np.empty((S, N, C), np.float32)
        lo = max(0, -t0)  # steps with t < 0 (core 0 only)
        if lo:
            xw[:lo] = x_star[None, None, :]
        xw[lo:] = x[t0 + lo : t0 + S]
        xwT = xw.transpose(2, 0, 1)  # (C, S, N)
        xTb = np.ascontiguousarray(
            xwT[:, :NBF].reshape(C, NBF * N).astype(ml_dtypes.bfloat16)
        )
        xT = np.ascontiguousarray(xwT[:, NBF:].reshape(C, (S - NBF) * N))
        in_maps.append(
            {
                "xT": xT,
                "xTb": xTb,
                "wxb": wxb,
                "wxt": wxt,
                "wht": wht,
                "wyt": wyt,
                "bx": bxc,
                "by": byc,
            }
        )
    return in_maps


def _assemble(results):
    """Untranspose per-core (K, OWN*N) / (D, OWN*N) outputs into full
    (T, N, K) / (T, N, D) arrays."""
    y_full = np.empty((T, N, K), np.float32)
    h_full = np.empty((T, N, D), np.float32)
    for i in range(NCORES):
        sl = slice(i * OWN, (i + 1) * OWN)
        y_full[sl] = (
            results[i]["y"].reshape(K, OWN, N).transpose(1, 2, 0)
        )
        h_full[sl] = (
            results[i]["h"].reshape(D, OWN, N).transpose(1, 2, 0)
        )
    return y_full, h_full


def _run(in_maps, trace=False, repeats=1):
    from concourse.bass_utils import run_bass_kernel_spmd

    nc = _get_program(repeats)
    return run_bass_kernel_spmd(
        nc, in_maps, list(range(NCORES)), trace=trace
    )


def kernel(x, W_x, b_x, W_h, W_y, b_y):
    in_maps = _prep_inputs(x, W_x, b_x, W_h, W_y, b_y)
    res = _run(in_maps)
    return _assemble(res.results)



# revision 2
# speedup vs baseline: 2.4296x; 2.4296x over previous
"""Elman RNN on 8 Trainium2 NeuronCores.

Strategy: time-shard T=512 into 16 segments of 32 owned steps; each core
runs TWO segments ("chains" alpha/beta) interleaved so the serial
relu->matmul dependency of one chain hides the other's latency. Each
chain re-runs a 16-step burn-in from h=0 before its owned window — the
relu recurrence is contractive (~0.74/step), so the state converges to
well below the bf16 noise floor. Segment 0 has no real predecessor
steps; its burn-in input is a forcing vector x* with W_x @ x* = -1e4, so
relu clamps h to exactly 0 until its window starts.

Everything runs in bf16 (matmuls are 1 cycle/row vs 4 for fp32; I/O
halves): weights, x, g = relu state, and both outputs; PSUM accumulates
f32. CPU emulation puts the end-to-end error at ~5e-3 vs the 2e-2 gate.

On-chip layout is transposed: g = h^T lives as (D=128 partitions,
N=256 free) per step. Per chain per step:
  PE:   psum_pair[:, step] += W_h^T.T @ g_prev   (xproj pre-filled per pair)
  ACT (chain a) / DVE (chain b): g = relu(psum + b_x), full 256 cols, bf16 out
Owned steps: y^T = W_y^T.T @ g into a quad PSUM tile, evacuated per quad
(ACT for chain a, DVE for chain b) with b_y added, DMA'd bf16. h^T is
DMA'd straight from the g quads. Host untransposes + upcasts to f32.
"""

import sys

if "/opt/trn_rl_repo" not in sys.path:
    sys.path.insert(0, "/opt/trn_rl_repo")

import numpy as np

T, N, C, D, K = 512, 256, 128, 128, 128
NCORES = 8
NCH = 2                    # interleaved chains (time segments) per core
OWN = T // (NCORES * NCH)  # 32 owned timesteps per chain
BURN = 16                  # burn-in steps (contraction reaches bf16 floor)
S = OWN + BURN             # 48 recurrence steps per chain
FORCE = 1.0e4
QPF = 3                    # x-quad DMA prefetch depth
PAIRS = S // 2
QUADS = S // 4

_prog_cache = {}


def _build_program():
    from contextlib import ExitStack

    import concourse.tile as tile
    from concourse import bacc, mybir

    f32 = mybir.dt.float32
    bf = mybir.dt.bfloat16
    AF = mybir.ActivationFunctionType
    ALU = mybir.AluOpType

    nc = bacc.Bacc(
        "TRN2", target_bir_lowering=False, debug=False, num_devices=NCORES
    )
    x_in = [
        nc.dram_tensor(f"x{c}", [C, S * N], bf, kind="ExternalInput").ap()
        for c in range(NCH)
    ]
    wxb = nc.dram_tensor("wxb", [C, D], bf, kind="ExternalInput").ap()
    whb = nc.dram_tensor("whb", [D, D], bf, kind="ExternalInput").ap()
    wyb = nc.dram_tensor("wyb", [D, K], bf, kind="ExternalInput").ap()
    bx = nc.dram_tensor("bx", [D, 1], f32, kind="ExternalInput").ap()
    by = nc.dram_tensor("by", [K, 1], f32, kind="ExternalInput").ap()
    y_o = [
        nc.dram_tensor(f"y{c}", [K, OWN * N], bf, kind="ExternalOutput").ap()
        for c in range(NCH)
    ]
    h_o = [
        nc.dram_tensor(f"h{c}", [D, OWN * N], bf, kind="ExternalOutput").ap()
        for c in range(NCH)
    ]

    with ExitStack() as ctx:
        tc = ctx.enter_context(tile.TileContext(nc))
        consts = ctx.enter_context(tc.tile_pool(name="consts", bufs=1))
        xtp = [
            ctx.enter_context(tc.tile_pool(name=f"xt{c}", bufs=QPF + 1))
            for c in range(NCH)
        ]
        gqp = [
            ctx.enter_context(tc.tile_pool(name=f"gq{c}", bufs=3))
            for c in range(NCH)
        ]
        styp = [
            ctx.enter_context(tc.tile_pool(name=f"sty{c}", bufs=2))
            for c in range(NCH)
        ]
        recp = [
            ctx.enter_context(tc.tile_pool(name=f"rec{c}", bufs=2, space="PSUM"))
            for c in range(NCH)
        ]
        yqp = [
            ctx.enter_context(tc.tile_pool(name=f"yq{c}", bufs=1, space="PSUM"))
            for c in range(NCH)
        ]

        wxb_sb = consts.tile([C, D], bf)
        nc.sync.dma_start(wxb_sb[:], wxb)
        whb_sb = consts.tile([D, D], bf)
        nc.sync.dma_start(whb_sb[:], whb)
        wyb_sb = consts.tile([D, K], bf)
        nc.sync.dma_start(wyb_sb[:], wyb)
        bx_sb = consts.tile([D, 1], f32)
        nc.sync.dma_start(bx_sb[:], bx)
        by_sb = consts.tile([K, 1], f32)
        nc.sync.dma_start(by_sb[:], by)

        xq_tiles = [{} for _ in range(NCH)]
        rec_tiles = [{} for _ in range(NCH)]
        gq_tiles = [{} for _ in range(NCH)]
        yq_tiles = [None] * NCH
        pend = [None] * NCH

        def emit_xdma(c, q):
            if q >= QUADS:
                return
            t = xtp[c].tile([C, 4 * N], bf, name=f"xt{c}_t", tag=f"xt{c}_t")
            nc.sync.dma_start(t[:], x_in[c][:, q * 4 * N : (q + 1) * 4 * N])
            xq_tiles[c][q] = t

        def emit_xproj(c, p):
            """Pre-fill the pair-(p) rec PSUM tile with W_x^T.T @ x."""
            if p >= PAIRS:
                return
            q, h2 = divmod(p, 2)
            xt = xq_tiles[c][q]
            r = recp[c].tile([D, 2 * N], f32, name=f"rec{c}_t", tag=f"rec{c}_t")
            nc.tensor.matmul(
                r[:],
                wxb_sb[:],
                xt[:, h2 * 2 * N : (h2 + 1) * 2 * N],
                start=True,
                stop=True,
            )
            rec_tiles[c][p] = r
            if h2 == 1:
                del xq_tiles[c][q]

        def emit_y(c, s, g_sl):
            """Deferred y^T matmul for step s, plus per-quad evac+DMA."""
            if s < BURN:
                return
            o = s - BURN
            oq, e = divmod(o, 4)
            if e == 0:
                yq_tiles[c] = yqp[c].tile(
                    [K, 4 * N], f32, name=f"yq{c}_t", tag=f"yq{c}_t"
                )
            yq = yq_tiles[c]
            # has_written clearing is per PSUM bank; the quad tile spans two
            # banks (slices 0-1 and 2-3), so the first slice landing in each
            # bank opens/closes that bank's group and the second overwrites
            # via the cleared has_written bits.
            opener = e % 2 == 0
            nc.tensor.matmul(
                yq[:, e * N : (e + 1) * N],
                wyb_sb[:],
                g_sl,
                start=opener,
                stop=opener,
                skip_group_check=not opener,
            )
            if e == 3:
                sty = styp[c].tile(
                    [K, 4 * N], bf, name=f"sty{c}_t", tag=f"sty{c}_t"
                )
                if c == 0:
                    nc.scalar.activation(
                        sty[:], yq[:], AF.Identity, bias=by_sb[:]
                    )
                else:
                    nc.vector.tensor_scalar_add(sty[:], yq[:], by_sb[:])
                nc.gpsimd.dma_start(
                    y_o[c][:, oq * 4 * N : (oq + 1) * 4 * N], sty[:]
                )

        for c in range(NCH):
            for q in range(QPF):
                emit_xdma(c, q)
        for c in range(NCH):
            emit_xproj(c, 0)

        for s in range(S):
            p, e2 = divmod(s, 2)
            quad, e4 = divmod(s, 4)
            if e4 == 0:
                for c in range(NCH):
                    emit_xdma(c, quad + QPF)
            if e2 == 0:
                for c in range(NCH):
                    emit_xproj(c, p + 1)
            for c in range(NCH):
                if pend[c] is not None:
                    emit_y(c, *pend[c])
            for c in range(NCH):
                if s > 0:
                    pq, pe = divmod(s - 1, 4)
                    gp = gq_tiles[c][pq]
                    nc.tensor.matmul(
                        rec_tiles[c][p][:, e2 * N : (e2 + 1) * N],
                        whb_sb[:],
                        gp[:, pe * N : (pe + 1) * N],
                        start=False,
                        stop=False,
                        skip_group_check=True,
                    )
            for c in range(NCH):
                if e4 == 0:
                    gq_tiles[c][quad] = gqp[c].tile(
                        [D, 4 * N], bf, name=f"gq{c}_t", tag=f"gq{c}_t"
                    )
                gq = gq_tiles[c][quad]
                rec_sl = rec_tiles[c][p][:, e2 * N : (e2 + 1) * N]
                g_sl = gq[:, e4 * N : (e4 + 1) * N]
                if c == 0:
                    nc.scalar.activation(g_sl, rec_sl, AF.Relu, bias=bx_sb[:])
                else:
                    nc.vector.tensor_scalar(
                        g_sl, rec_sl, bx_sb[:], 0.0, ALU.add, ALU.max
                    )
                pend[c] = (s, g_sl)
                if e4 == 3 and s >= BURN:
                    oq = quad - BURN // 4
                    nc.gpsimd.dma_start(
                        h_o[c][:, oq * 4 * N : (oq + 1) * 4 * N], gq[:]
                    )
                if e4 == 3 and quad - 1 in gq_tiles[c]:
                    del gq_tiles[c][quad - 1]
                if e2 == 1:
                    rec_tiles[c].pop(p, None)
        for c in range(NCH):
            emit_y(c, *pend[c])

    nc.compile()
    return nc


def _get_program():
    if "p" not in _prog_cache:
        _prog_cache["p"] = _build_program()
    return _prog_cache["p"]


def _prep_inputs(x, W_x, b_x, W_h, W_y, b_y):
    import ml_dtypes

    bf16 = ml_dtypes.bfloat16

    x = np.ascontiguousarray(x, np.float32)
    W_x = np.asarray(W_x, np.float32)
    b_x = np.asarray(b_x, np.float32)
    W_h = np.asarray(W_h, np.float32)
    W_y = np.asarray(W_y, np.float32)
    b_y = np.asarray(b_y, np.float32)

    # segment-0 burn-in forcing vector: W_x @ x_star = -FORCE (relu clamps
    # the state to exactly 0 through the fake burn-in steps)
    lam = np.linalg.solve(
        W_x.astype(np.float64) @ W_x.astype(np.float64).T,
        -FORCE * np.ones(D, np.float64),
    )
    x_star = (W_x.astype(np.float64).T @ lam).astype(np.float32)

    wxb = np.ascontiguousarray(W_x.T).astype(bf16)     # (C, D)
    whb = np.ascontiguousarray(W_h.T).astype(bf16)     # (D, D)
    wyb = np.ascontiguousarray(W_y.T).astype(bf16)     # (D, K)
    bxc = np.ascontiguousarray(b_x[:, None])           # (D, 1)
    byc = np.ascontiguousarray(b_y[:, None])           # (K, 1)

    xbf = x.astype(bf16)
    xstar_bf = x_star.astype(bf16)

    in_maps = []
    for core in range(NCORES):
        m = {"wxb": wxb, "whb": whb, "wyb": wyb, "bx": bxc, "by": byc}
        for c in range(NCH):
            t0 = (core * NCH + c) * OWN - BURN
            xw = np.empty((S, N, C), bf16)
            lo = max(0, -t0)  # steps with t < 0 (segment 0 only)
            if lo:
                xw[:lo] = xstar_bf[None, None, :]
            xw[lo:] = xbf[t0 + lo : t0 + S]
            m[f"x{c}"] = np.ascontiguousarray(
                xw.transpose(2, 0, 1).reshape(C, S * N)
            )
        in_maps.append(m)
    return in_maps


def _assemble(results):
    """Untranspose per-core per-chain (K, OWN*N) / (D, OWN*N) bf16 outputs
    into full (T, N, K) / (T, N, D) f32 arrays."""
    y_full = np.empty((T, N, K), np.float32)
    h_full = np.empty((T, N, D), np.float32)
    for i in range(NCORES):
        for c in range(NCH):
            t0 = (i * NCH + c) * OWN
            sl = slice(t0, t0 + OWN)
            y_full[sl] = (
                results[i][f"y{c}"]
                .astype(np.float32)
                .reshape(K, OWN, N)
                .transpose(1, 2, 0)
            )
            h_full[sl] = (
                results[i][f"h{c}"]
                .astype(np.float32)
                .reshape(D, OWN, N)
                .transpose(1, 2, 0)
            )
    return y_full, h_full


def _run(in_maps, trace=False, repeats=1):
    from concourse.bass_utils import run_bass_kernel_spmd

    nc = _get_program()
    return run_bass_kernel_spmd(
        nc, in_maps, list(range(NCORES)), trace=trace
    )


def kernel(x, W_x, b_x, W_h, W_y, b_y):
    in_maps = _prep_inputs(x, W_x, b_x, W_h, W_y, b_y)
    res = _run(in_maps)
    return _assemble(res.results)


# revision 6
# speedup vs baseline: 2.4876x; 1.0239x over previous
"""Elman RNN on 8 Trainium2 NeuronCores.

Strategy: time-shard T=512 into 16 segments of 32 owned steps; each core
runs TWO segments ("chains" alpha/beta) interleaved so the serial
relu->matmul dependency of one chain hides the other's latency. Each
chain re-runs a 16-step burn-in from h=0 before its owned window — the
relu recurrence is contractive (~0.74/step), so the state converges to
well below the bf16 noise floor. Segment 0 has no real predecessor
steps; its burn-in input is a forcing vector x* with W_x @ x* = -1e4, so
relu clamps h to exactly 0 until its window starts.

Everything runs in bf16 (matmuls are 1 cycle/row vs 4 for fp32; I/O
halves): weights, x, g = relu state, and both outputs; PSUM accumulates
f32. CPU emulation puts the end-to-end error at ~5e-3 vs the 2e-2 gate.

On-chip layout is transposed: g = h^T lives as (D=128 partitions,
N=256 free) per step. Per chain per step:
  PE:   psum_pair[:, step] += W_h^T.T @ g_prev   (xproj pre-filled per pair)
  ACT (chain a) / DVE (chain b): g = relu(psum + b_x), full 256 cols, bf16 out
Owned steps: y^T = W_y^T.T @ g into a quad PSUM tile, evacuated per quad
(ACT for chain a, DVE for chain b) with b_y added, DMA'd bf16. h^T is
DMA'd straight from the g quads. Host untransposes + upcasts to f32.
"""

import sys

if "/opt/trn_rl_repo" not in sys.path:
    sys.path.insert(0, "/opt/trn_rl_repo")

import numpy as np

T, N, C, D, K = 512, 256, 128, 128, 128
NCORES = 8
NCH = 2                    # interleaved chains (time segments) per core
OWN = T // (NCORES * NCH)  # 32 owned timesteps per chain
BURN = 16                  # burn-in steps (contraction reaches bf16 floor)
S = OWN + BURN             # 48 recurrence steps per chain
FORCE = 1.0e4
QPF = 3                    # x-quad DMA prefetch depth
PAIRS = S // 2
QUADS = S // 4

_prog_cache = {}


def _build_program():
    from contextlib import ExitStack

    import concourse.tile as tile
    from concourse import bacc, mybir

    f32 = mybir.dt.float32
    bf = mybir.dt.bfloat16
    AF = mybir.ActivationFunctionType
    ALU = mybir.AluOpType

    nc = bacc.Bacc(
        "TRN2", target_bir_lowering=False, debug=False, num_devices=NCORES
    )
    x_in = [
        nc.dram_tensor(f"x{c}", [C, S * N], bf, kind="ExternalInput").ap()
        for c in range(NCH)
    ]
    wxb = nc.dram_tensor("wxb", [C, D], bf, kind="ExternalInput").ap()
    whb = nc.dram_tensor("whb", [D, D], bf, kind="ExternalInput").ap()
    wyb = nc.dram_tensor("wyb", [D, K], bf, kind="ExternalInput").ap()
    bx = nc.dram_tensor("bx", [D, 1], f32, kind="ExternalInput").ap()
    by = nc.dram_tensor("by", [K, 1], f32, kind="ExternalInput").ap()
    y_o = [
        nc.dram_tensor(f"y{c}", [K, OWN * N], bf, kind="ExternalOutput").ap()
        for c in range(NCH)
    ]
    h_o = [
        nc.dram_tensor(f"h{c}", [D, OWN * N], bf, kind="ExternalOutput").ap()
        for c in range(NCH)
    ]

    with ExitStack() as ctx:
        tc = ctx.enter_context(tile.TileContext(nc))
        consts = ctx.enter_context(tc.tile_pool(name="consts", bufs=1))
        xtp = [
            ctx.enter_context(tc.tile_pool(name=f"xt{c}", bufs=QPF + 1))
            for c in range(NCH)
        ]
        gqp = [
            ctx.enter_context(tc.tile_pool(name=f"gq{c}", bufs=3))
            for c in range(NCH)
        ]
        styp = [
            ctx.enter_context(tc.tile_pool(name=f"sty{c}", bufs=2))
            for c in range(NCH)
        ]
        recp = [
            ctx.enter_context(tc.tile_pool(name=f"rec{c}", bufs=2, space="PSUM"))
            for c in range(NCH)
        ]
        yqp = [
            ctx.enter_context(tc.tile_pool(name=f"yq{c}", bufs=1, space="PSUM"))
            for c in range(NCH)
        ]
        filp = ctx.enter_context(tc.tile_pool(name="fil", bufs=1, space="PSUM"))

        wxb_sb = consts.tile([C, D], bf)
        nc.sync.dma_start(wxb_sb[:], wxb)
        whb_sb = consts.tile([D, D], bf)
        nc.sync.dma_start(whb_sb[:], whb)
        wyb_sb = consts.tile([D, K], bf)
        nc.sync.dma_start(wyb_sb[:], wyb)
        bx_sb = consts.tile([D, 1], f32)
        nc.sync.dma_start(bx_sb[:], bx)
        by_sb = consts.tile([K, 1], f32)
        nc.sync.dma_start(by_sb[:], by)

        # HAM keep-warm filler: a 1-output-row bf16 matmul streaming columns
        # keeps the PE array "busy" through the per-step relu windows, so the
        # clock gate stays at 2.4 GHz instead of re-throttling to 1.2 GHz
        # (which doubles every real matmul).
        fill_w = consts.tile([D, 1], bf)
        nc.vector.memset(fill_w[:], 0.0)
        fill_x = consts.tile([D, 2 * N], bf)
        nc.vector.memset(fill_x[:], 0.0)
        fil_ps = filp.tile([1, 2 * N], f32)

        def emit_filler(ncols):
            nc.tensor.matmul(
                fil_ps[0:1, 0:ncols],
                fill_w[:],
                fill_x[:, 0:ncols],
                start=True,
                stop=True,
            )

        xq_tiles = [{} for _ in range(NCH)]
        rec_tiles = [{} for _ in range(NCH)]
        gq_tiles = [{} for _ in range(NCH)]
        yq_tiles = [None] * NCH
        pend = [None] * NCH

        def emit_xdma(c, q):
            if q >= QUADS:
                return
            t = xtp[c].tile([C, 4 * N], bf, name=f"xt{c}_t", tag=f"xt{c}_t")
            nc.sync.dma_start(t[:], x_in[c][:, q * 4 * N : (q + 1) * 4 * N])
            xq_tiles[c][q] = t

        def emit_xproj(c, p):
            """Pre-fill the pair-(p) rec PSUM tile with W_x^T.T @ x."""
            if p >= PAIRS:
                return
            q, h2 = divmod(p, 2)
            xt = xq_tiles[c][q]
            r = recp[c].tile([D, 2 * N], f32, name=f"rec{c}_t", tag=f"rec{c}_t")
            nc.tensor.matmul(
                r[:],
                wxb_sb[:],
                xt[:, h2 * 2 * N : (h2 + 1) * 2 * N],
                start=True,
                stop=True,
            )
            rec_tiles[c][p] = r
            if h2 == 1:
                del xq_tiles[c][q]

        sty_tiles = [None] * NCH

        def emit_y(c, s, g_sl):
            """Deferred y^T matmul for step s; evac per pair into a quad
            staging tile, DMA per quad."""
            if s < BURN:
                return
            o = s - BURN
            oq, e4 = divmod(o, 4)
            e = o % 2
            if e == 0:
                yq_tiles[c] = yqp[c].tile(
                    [K, 2 * N], f32, name=f"yq{c}_t", tag=f"yq{c}_t"
                )
            yq = yq_tiles[c]
            # has_written clearing is per PSUM bank: the first slice opens and
            # closes the bank's group, the second overwrites via the cleared
            # has_written bits.
            nc.tensor.matmul(
                yq[:, e * N : (e + 1) * N],
                wyb_sb[:],
                g_sl,
                start=e == 0,
                stop=e == 0,
                skip_group_check=e == 1,
            )
            if e == 1:
                if e4 == 1:
                    sty_tiles[c] = styp[c].tile(
                        [K, 4 * N], bf, name=f"sty{c}_t", tag=f"sty{c}_t"
                    )
                sty = sty_tiles[c]
                half = (e4 - 1) // 2  # 0 for pair 0-1, 1 for pair 2-3
                sty_sl = sty[:, half * 2 * N : (half + 1) * 2 * N]
                if c == 0:
                    nc.scalar.activation(
                        sty_sl, yq[:], AF.Identity, bias=by_sb[:]
                    )
                else:
                    nc.vector.tensor_scalar_add(sty_sl, yq[:], by_sb[:])
                if e4 == 3:
                    nc.gpsimd.dma_start(
                        y_o[c][:, oq * 4 * N : (oq + 1) * 4 * N], sty[:]
                    )

        for c in range(NCH):
            for q in range(QPF):
                emit_xdma(c, q)
        for c in range(NCH):
            emit_xproj(c, 0)

        for s in range(S):
            p, e2 = divmod(s, 2)
            quad, e4 = divmod(s, 4)
            if e4 == 0:
                for c in range(NCH):
                    emit_xdma(c, quad + QPF)
            if e2 == 0:
                for c in range(NCH):
                    emit_xproj(c, p + 1)
            for c in range(NCH):
                if pend[c] is not None:
                    emit_y(c, *pend[c])
            for c in range(NCH):
                if s > 0:
                    pq, pe = divmod(s - 1, 4)
                    gp = gq_tiles[c][pq]
                    nc.tensor.matmul(
                        rec_tiles[c][p][:, e2 * N : (e2 + 1) * N],
                        whb_sb[:],
                        gp[:, pe * N : (pe + 1) * N],
                        start=False,
                        stop=False,
                        skip_group_check=True,
                    )
            for _f in range(2):
                emit_filler(2 * N)
            for c in range(NCH):
                if e4 == 0:
                    gq_tiles[c][quad] = gqp[c].tile(
                        [D, 4 * N], bf, name=f"gq{c}_t", tag=f"gq{c}_t"
                    )
                gq = gq_tiles[c][quad]
                rec_sl = rec_tiles[c][p][:, e2 * N : (e2 + 1) * N]
                g_sl = gq[:, e4 * N : (e4 + 1) * N]
                if c == 0:
                    nc.scalar.activation(g_sl, rec_sl, AF.Relu, bias=bx_sb[:])
                else:
                    nc.vector.tensor_scalar(
                        g_sl, rec_sl, bx_sb[:], 0.0, ALU.add, ALU.max
                    )
                pend[c] = (s, g_sl)
                if e4 == 3 and s >= BURN:
                    oq = quad - BURN // 4
                    nc.gpsimd.dma_start(
                        h_o[c][:, oq * 4 * N : (oq + 1) * 4 * N], gq[:]
                    )
                if e4 == 3 and quad - 1 in gq_tiles[c]:
                    del gq_tiles[c][quad - 1]
                if e2 == 1:
                    rec_tiles[c].pop(p, None)
        for c in range(NCH):
            emit_y(c, *pend[c])

    nc.compile()
    return nc


def _get_program():
    if "p" not in _prog_cache:
        _prog_cache["p"] = _build_program()
    return _prog_cache["p"]


def _prep_inputs(x, W_x, b_x, W_h, W_y, b_y):
    import ml_dtypes

    bf16 = ml_dtypes.bfloat16

    x = np.ascontiguousarray(x, np.float32)
    W_x = np.asarray(W_x, np.float32)
    b_x = np.asarray(b_x, np.float32)
    W_h = np.asarray(W_h, np.float32)
    W_y = np.asarray(W_y, np.float32)
    b_y = np.asarray(b_y, np.float32)

    # segment-0 burn-in forcing vector: W_x @ x_star = -FORCE (relu clamps
    # the state to exactly 0 through the fake burn-in steps)
    lam = np.linalg.solve(
        W_x.astype(np.float64) @ W_x.astype(np.float64).T,
        -FORCE * np.ones(D, np.float64),
    )
    x_star = (W_x.astype(np.float64).T @ lam).astype(np.float32)

    wxb = np.ascontiguousarray(W_x.T).astype(bf16)     # (C, D)
    whb = np.ascontiguousarray(W_h.T).astype(bf16)     # (D, D)
    wyb = np.ascontiguousarray(W_y.T).astype(bf16)     # (D, K)
    bxc = np.ascontiguousarray(b_x[:, None])           # (D, 1)
    byc = np.ascontiguousarray(b_y[:, None])           # (K, 1)

    xbf = x.astype(bf16)
    xstar_bf = x_star.astype(bf16)

    in_maps = []
    for core in range(NCORES):
        m = {"wxb": wxb, "whb": whb, "wyb": wyb, "bx": bxc, "by": byc}
        for c in range(NCH):
            t0 = (core * NCH + c) * OWN - BURN
            xw = np.empty((S, N, C), bf16)
            lo = max(0, -t0)  # steps with t < 0 (segment 0 only)
            if lo:
                xw[:lo] = xstar_bf[None, None, :]
            xw[lo:] = xbf[t0 + lo : t0 + S]
            m[f"x{c}"] = np.ascontiguousarray(
                xw.transpose(2, 0, 1).reshape(C, S * N)
            )
        in_maps.append(m)
    return in_maps


def _assemble(results):
    """Untranspose per-core per-chain (K, OWN*N) / (D, OWN*N) bf16 outputs
    into full (T, N, K) / (T, N, D) f32 arrays."""
    y_full = np.empty((T, N, K), np.float32)
    h_full = np.empty((T, N, D), np.float32)
    for i in range(NCORES):
        for c in range(NCH):
            t0 = (i * NCH + c) * OWN
            sl = slice(t0, t0 + OWN)
            y_full[sl] = (
                results[i][f"y{c}"]
                .astype(np.float32)
                .reshape(K, OWN, N)
                .transpose(1, 2, 0)
            )
            h_full[sl] = (
                results[i][f"h{c}"]
                .astype(np.float32)
                .reshape(D, OWN, N)
                .transpose(1, 2, 0)
            )
    return y_full, h_full


def _run(in_maps, trace=False, repeats=1):
    from concourse.bass_utils import run_bass_kernel_spmd

    nc = _get_program()
    return run_bass_kernel_spmd(
        nc, in_maps, list(range(NCORES)), trace=trace
    )


def kernel(x, W_x, b_x, W_h, W_y, b_y):
    in_maps = _prep_inputs(x, W_x, b_x, W_h, W_y, b_y)
    res = _run(in_maps)
    return _assemble(res.results)


# revision 11
# speedup vs baseline: 2.9508x; 1.1862x over previous
"""Elman RNN on 8 Trainium2 NeuronCores.

Strategy: time-shard T=512 into 16 segments of 32 owned steps; each core
runs TWO segments ("chains" alpha/beta) interleaved so the serial
relu->matmul dependency of one chain hides the other's latency. Each
chain re-runs a 16-step burn-in from h=0 before its owned window — the
relu recurrence is contractive (~0.74/step), so the state converges to
well below the bf16 noise floor. Segment 0 has no real predecessor
steps; its burn-in input is a forcing vector x* with W_x @ x* = -1e4, so
relu clamps h to exactly 0 until its window starts.

Everything runs in bf16 (matmuls are 1 cycle/row vs 4 for fp32; I/O
halves): weights, x, g = relu state, and both outputs; PSUM accumulates
f32. CPU emulation puts the end-to-end error at ~5e-3 vs the 2e-2 gate.

On-chip layout is transposed: g = h^T lives as (D=128 partitions,
N=256 free) per step. Per chain per step:
  PE:   psum_pair[:, step] += W_h^T.T @ g_prev   (xproj pre-filled per pair)
  ACT (chain a) / DVE (chain b): g = relu(psum + b_x), full 256 cols, bf16 out
Owned steps: y^T = W_y^T.T @ g into a quad PSUM tile, evacuated per quad
(ACT for chain a, DVE for chain b) with b_y added, DMA'd bf16. h^T is
DMA'd straight from the g quads. Host untransposes + upcasts to f32.
"""

import sys

if "/opt/trn_rl_repo" not in sys.path:
    sys.path.insert(0, "/opt/trn_rl_repo")

import numpy as np

T, N, C, D, K = 512, 256, 128, 128, 128
NCORES = 8
NCH = 2                    # interleaved chains (time segments) per core
OWN = T // (NCORES * NCH)  # 32 owned timesteps per chain
BURN = 16                  # burn-in steps (contraction reaches bf16 floor)
S = OWN + BURN             # 48 recurrence steps per chain
FORCE = 1.0e4
QPF = 3                    # x-quad DMA prefetch depth
PAIRS = S // 2
QUADS = S // 4

_prog_cache = {}


def _build_program():
    from contextlib import ExitStack

    import concourse.tile as tile
    from concourse import bacc, mybir

    f32 = mybir.dt.float32
    bf = mybir.dt.bfloat16
    AF = mybir.ActivationFunctionType
    ALU = mybir.AluOpType

    nc = bacc.Bacc(
        "TRN2", target_bir_lowering=False, debug=False, num_devices=NCORES
    )
    x_in = [
        nc.dram_tensor(f"x{c}", [C, S * N], bf, kind="ExternalInput").ap()
        for c in range(NCH)
    ]
    wxb = nc.dram_tensor("wxb", [C, D], bf, kind="ExternalInput").ap()
    whb = nc.dram_tensor("whb", [D, D], bf, kind="ExternalInput").ap()
    wyb = nc.dram_tensor("wyb", [D, K], bf, kind="ExternalInput").ap()
    bx = nc.dram_tensor("bx", [D, 1], f32, kind="ExternalInput").ap()
    by = nc.dram_tensor("by", [K, 1], f32, kind="ExternalInput").ap()
    y_o = [
        nc.dram_tensor(f"y{c}", [K, OWN * N], bf, kind="ExternalOutput").ap()
        for c in range(NCH)
    ]
    h_o = [
        nc.dram_tensor(f"h{c}", [D, OWN * N], bf, kind="ExternalOutput").ap()
        for c in range(NCH)
    ]

    with ExitStack() as ctx:
        tc = ctx.enter_context(tile.TileContext(nc))
        consts = ctx.enter_context(tc.tile_pool(name="consts", bufs=1))
        xtp = [
            ctx.enter_context(tc.tile_pool(name=f"xt{c}", bufs=QPF + 1))
            for c in range(NCH)
        ]
        gqp = [
            ctx.enter_context(tc.tile_pool(name=f"gq{c}", bufs=3))
            for c in range(NCH)
        ]
        styp = [
            ctx.enter_context(tc.tile_pool(name=f"sty{c}", bufs=2))
            for c in range(NCH)
        ]
        recp = [
            ctx.enter_context(tc.tile_pool(name=f"rec{c}", bufs=3, space="PSUM"))
            for c in range(NCH)
        ]
        yqp = [
            ctx.enter_context(tc.tile_pool(name=f"yq{c}", bufs=1, space="PSUM"))
            for c in range(NCH)
        ]

        wxb_sb = consts.tile([C, D], bf)
        nc.sync.dma_start(wxb_sb[:], wxb)
        whb_sb = consts.tile([D, D], bf)
        nc.sync.dma_start(whb_sb[:], whb)
        wyb_sb = consts.tile([D, K], bf)
        nc.sync.dma_start(wyb_sb[:], wyb)
        bx_sb = consts.tile([D, 1], f32)
        nc.sync.dma_start(bx_sb[:], bx)
        by_sb = consts.tile([K, 1], f32)
        nc.sync.dma_start(by_sb[:], by)

        xq_tiles = [{} for _ in range(NCH)]
        rec_tiles = [{} for _ in range(NCH)]
        gq_tiles = [{} for _ in range(NCH)]
        yq_tiles = [None] * NCH
        pend = [None] * NCH

        def emit_xdma(c, q):
            if q >= QUADS:
                return
            t = xtp[c].tile([C, 4 * N], bf, name=f"xt{c}_t", tag=f"xt{c}_t")
            nc.sync.dma_start(t[:], x_in[c][:, q * 4 * N : (q + 1) * 4 * N])
            xq_tiles[c][q] = t

        def emit_xproj(c, p):
            """Pre-fill the pair-(p) rec PSUM tile with W_x^T.T @ x."""
            if p >= PAIRS:
                return
            q, h2 = divmod(p, 2)
            xt = xq_tiles[c][q]
            r = recp[c].tile([D, 2 * N], f32, name=f"rec{c}_t", tag=f"rec{c}_t")
            nc.tensor.matmul(
                r[:],
                wxb_sb[:],
                xt[:, h2 * 2 * N : (h2 + 1) * 2 * N],
                start=True,
                stop=True,
            )
            rec_tiles[c][p] = r
            if h2 == 1:
                del xq_tiles[c][q]

        sty_tiles = [None] * NCH

        def emit_y(c, s, g_sl):
            """Deferred y^T pair matmul for owned steps (s-1, s): one 512-col
            matmul into a 1-bank PSUM tile, evac per pair into a quad staging
            tile (ACT for chain 0, Pool for chain 1), DMA per quad."""
            if s < BURN:
                return
            o = s - BURN          # odd: pair covers o-1, o
            oq, e4 = divmod(o, 4)
            yq = yqp[c].tile([K, 2 * N], f32, name=f"yq{c}_t", tag=f"yq{c}_t")
            nc.tensor.matmul(yq[:], wyb_sb[:], g_sl, start=True, stop=True)
            if e4 == 1:
                sty_tiles[c] = styp[c].tile(
                    [K, 4 * N], bf, name=f"sty{c}_t", tag=f"sty{c}_t"
                )
            sty = sty_tiles[c]
            half = (e4 - 1) // 2  # 0 for steps 0-1, 1 for steps 2-3
            sty_sl = sty[:, half * 2 * N : (half + 1) * 2 * N]
            if c == 0:
                nc.scalar.activation(sty_sl, yq[:], AF.Identity, bias=by_sb[:])
            else:
                nc.vector.tensor_scalar_add(sty_sl, yq[:], by_sb[:])
            if e4 == 3:
                nc.gpsimd.dma_start(
                    y_o[c][:, oq * 4 * N : (oq + 1) * 4 * N], sty[:]
                )

        for c in range(NCH):
            for q in range(QPF):
                emit_xdma(c, q)
        for c in range(NCH):
            emit_xproj(c, 0)
            emit_xproj(c, 1)

        for s in range(S):
            p, e2 = divmod(s, 2)
            quad, e4 = divmod(s, 4)
            if e4 == 0:
                for c in range(NCH):
                    emit_xdma(c, quad + QPF)
            # rec matmuls FIRST in the PE stream: nothing may sit between the
            # relu-completion semaphore and the next step's recurrence.
            for c in range(NCH):
                if s > 0:
                    pq, pe = divmod(s - 1, 4)
                    gp = gq_tiles[c][pq]
                    nc.tensor.matmul(
                        rec_tiles[c][p][:, e2 * N : (e2 + 1) * N],
                        whb_sb[:],
                        gp[:, pe * N : (pe + 1) * N],
                        start=False,
                        stop=False,
                        skip_group_check=True,
                    )
            for c in range(NCH):
                if pend[c] is not None:
                    emit_y(c, *pend[c])
                    pend[c] = None
            if e2 == 0:
                for c in range(NCH):
                    emit_xproj(c, p + 2)
            for c in range(NCH):
                if e4 == 0:
                    gq_tiles[c][quad] = gqp[c].tile(
                        [D, 4 * N], bf, name=f"gq{c}_t", tag=f"gq{c}_t"
                    )
                gq = gq_tiles[c][quad]
                rec_sl = rec_tiles[c][p][:, e2 * N : (e2 + 1) * N]
                g_sl = gq[:, e4 * N : (e4 + 1) * N]
                if c == 0:
                    nc.scalar.activation(g_sl, rec_sl, AF.Relu, bias=bx_sb[:])
                else:
                    nc.vector.tensor_scalar(
                        g_sl, rec_sl, bx_sb[:], 0.0, ALU.add, ALU.max
                    )
                if e2 == 1:
                    pend[c] = (s, gq[:, (e4 - 1) * N : (e4 + 1) * N])
                if e4 == 3 and s >= BURN:
                    oq = quad - BURN // 4
                    nc.gpsimd.dma_start(
                        h_o[c][:, oq * 4 * N : (oq + 1) * 4 * N], gq[:]
                    )
                if e4 == 3 and quad - 1 in gq_tiles[c]:
                    del gq_tiles[c][quad - 1]
                if e2 == 1:
                    rec_tiles[c].pop(p, None)
        for c in range(NCH):
            emit_y(c, *pend[c])

    nc.compile()
    return nc


def _get_program():
    if "p" not in _prog_cache:
        _prog_cache["p"] = _build_program()
    return _prog_cache["p"]


def _prep_inputs(x, W_x, b_x, W_h, W_y, b_y):
    import ml_dtypes

    bf16 = ml_dtypes.bfloat16

    x = np.ascontiguousarray(x, np.float32)
    W_x = np.asarray(W_x, np.float32)
    b_x = np.asarray(b_x, np.float32)
    W_h = np.asarray(W_h, np.float32)
    W_y = np.asarray(W_y, np.float32)
    b_y = np.asarray(b_y, np.float32)

    # segment-0 burn-in forcing vector: W_x @ x_star = -FORCE (relu clamps
    # the state to exactly 0 through the fake burn-in steps)
    lam = np.linalg.solve(
        W_x.astype(np.float64) @ W_x.astype(np.float64).T,
        -FORCE * np.ones(D, np.float64),
    )
    x_star = (W_x.astype(np.float64).T @ lam).astype(np.float32)

    wxb = np.ascontiguousarray(W_x.T).astype(bf16)     # (C, D)
    whb = np.ascontiguousarray(W_h.T).astype(bf16)     # (D, D)
    wyb = np.ascontiguousarray(W_y.T).astype(bf16)     # (D, K)
    bxc = np.ascontiguousarray(b_x[:, None])           # (D, 1)
    byc = np.ascontiguousarray(b_y[:, None])           # (K, 1)

    xbf = x.astype(bf16)
    xstar_bf = x_star.astype(bf16)

    in_maps = []
    for core in range(NCORES):
        m = {"wxb": wxb, "whb": whb, "wyb": wyb, "bx": bxc, "by": byc}
        for c in range(NCH):
            t0 = (core * NCH + c) * OWN - BURN
            xw = np.empty((S, N, C), bf16)
            lo = max(0, -t0)  # steps with t < 0 (segment 0 only)
            if lo:
                xw[:lo] = xstar_bf[None, None, :]
            xw[lo:] = xbf[t0 + lo : t0 + S]
            m[f"x{c}"] = np.ascontiguousarray(
                xw.transpose(2, 0, 1).reshape(C, S * N)
            )
        in_maps.append(m)
    return in_maps


def _assemble(results):
    """Untranspose per-core per-chain (K, OWN*N) / (D, OWN*N) bf16 outputs
    into full (T, N, K) / (T, N, D) f32 arrays."""
    y_full = np.empty((T, N, K), np.float32)
    h_full = np.empty((T, N, D), np.float32)
    for i in range(NCORES):
        for c in range(NCH):
            t0 = (i * NCH + c) * OWN
            sl = slice(t0, t0 + OWN)
            y_full[sl] = (
                results[i][f"y{c}"]
                .astype(np.float32)
                .reshape(K, OWN, N)
                .transpose(1, 2, 0)
            )
            h_full[sl] = (
                results[i][f"h{c}"]
                .astype(np.float32)
                .reshape(D, OWN, N)
                .transpose(1, 2, 0)
            )
    return y_full, h_full


def _run(in_maps, trace=False, repeats=1):
    from concourse.bass_utils import run_bass_kernel_spmd

    nc = _get_program()
    return run_bass_kernel_spmd(
        nc, in_maps, list(range(NCORES)), trace=trace
    )


def kernel(x, W_x, b_x, W_h, W_y, b_y):
    in_maps = _prep_inputs(x, W_x, b_x, W_h, W_y, b_y)
    res = _run(in_maps)
    return _assemble(res.results)
